# revision 36
# baseline (speedup 1.0000x reference)
"""Fused LoRA-Linear (per-token adapter routing) for 8 TRN2 NeuronCores.

Strategy:
  - Shard tokens: 8192 -> 1024 per core. Replicate weight/adapters.
    No cross-core communication (compute-regime problem).
  - Stack the 8 rank-16 adapters into one 128-row block:
        A_cat [128, 4096],  B_cat.T [128, 4096]
    Prologue per core: a_allT = A_cat @ x_shard^T  -> [128, 1024]  (PE)
    then ams = a_allT * smask where smask[j,t] = scal[t]*(idx[t]==j//16).
  - Main loop: out tile [128 tok, 512 dout] accumulates 32 base K-steps
    (lhsT = xT tile, rhs = W tile) plus ONE LoRA K-step
    (lhsT = ams column block, rhs = B_cat.T tile) in the same PSUM bank.
  - Drain: DVE adds broadcast bias while copying PSUM -> SBUF (bf16),
    DMA out; host converts back to f32.
  - Phase order: n=0 fuses the adapter prologue into its k-loop (PSUM:
    2 prologue + 6 base banks) so the x^T load streams concurrently with
    base matmuls; then n=1..7; the m=6 and m=7 re-sweeps of n=0 run LAST
    (m-outer, so m=6's drain hides under m=7's k-loop) on SBUF-resident
    W[0] tiles, hiding n=7's output-drain burst under their matmuls.
  - DMA batching: each DMA costs ~650ns SP-queue issue + ~625ns HWDGE +
    ~900ns semaphore propagation, so operands are host-packed into
    multi-k blocks (xT 2 k-steps, W/A 4 k-steps per DMA) and the n=0
    stream is issued one block ahead of first use; k>=31-only tensors
    (smask/B/bias) are issued behind the first two stream blocks.

Matmul operands are bf16 (fp32 PSUM accumulate): 1 PE cycle/row like
f32r but half the DMA bytes; ~3.0e-3 frobenius rel err vs the fp32
reference (harness gate is 2e-2).

Timeline-sim: 472.6us/core vs 464.2us pure-matmul floor (2176 matmuls
x 512 free-dim cycles @ 2.4GHz); PE.ENGINE busy is at the warm-clock
floor, the ~8.5us rest is the startup DMA latency chain + tail drain.
"""

import numpy as np

import concourse.bass as bass
import concourse.bacc as bacc
import concourse.mybir as mybir
import concourse.tile as tile
from concourse.bass_utils import run_bass_kernel_spmd

SEQ, D_IN, D_OUT, RANK, N_ADAPTERS = 8192, 4096, 4096, 16, 8
N_CORES = 8
T = SEQ // N_CORES          # 1024 tokens per core
P = 128                     # partitions
FD = 512                    # matmul free dim (one PSUM bank)
KO = D_IN // P              # 32 contraction tiles
NT = D_OUT // FD            # 8 output column chunks
MT = T // P                 # 8 token tiles per core
J = N_ADAPTERS * RANK       # 128 stacked adapter rows
XG = 2                      # k-steps per xT DMA
WG = 4                      # k-steps per W DMA
AG = 4                      # k-steps per A DMA
F32 = mybir.dt.float32
MMDT = mybir.dt.bfloat16  # matmul operand dtype (bf16: full-rate PE, half DMA)
NP_MMDT = mybir.dt.np(MMDT)
JC = 64                     # sorted path: adapter-window rows per core (4x16)

_NC_CACHE = {}


def _build_nc(reps=1):
    # reps>1 repeats the whole program in one NEFF (benchmarking only:
    # wall(T_R) - wall(T_1) cancels identical RPC/dispatch overheads)
    key = f"nc{reps}"
    if key in _NC_CACHE:
        return _NC_CACHE[key]
    nc = bacc.Bacc(None, target_bir_lowering=False, debug=False)
    xT = nc.dram_tensor("xT", [KO // XG, P, XG * T], MMDT, kind="ExternalInput")
    w = nc.dram_tensor("w", [NT, KO // WG, P, WG * FD], MMDT, kind="ExternalInput")
    biasb = nc.dram_tensor("biasb", [NT, P, FD], F32, kind="ExternalInput")
    at = nc.dram_tensor("at", [KO // AG, P, AG * J], MMDT, kind="ExternalInput")
    bt = nc.dram_tensor("bt", [NT, J, FD], MMDT, kind="ExternalInput")
    smask = nc.dram_tensor("smask", [J, T], F32, kind="ExternalInput")
    # bf16 output (host converts back to f32): halves the drain DMA bytes
    out = nc.dram_tensor("out", [T, D_OUT], MMDT, kind="ExternalOutput")

    with tile.TileContext(nc) as tc:
        with (
            tc.tile_pool(name="xt", bufs=1) as xt_pool,
            tc.tile_pool(name="w0", bufs=1) as w0_pool,
            tc.tile_pool(name="wp", bufs=4) as w_pool,
            tc.tile_pool(name="apool", bufs=3) as a_pool,
            tc.tile_pool(name="bp", bufs=2) as b_pool,
            tc.tile_pool(name="biasp", bufs=2) as bias_pool,
            tc.tile_pool(name="outp", bufs=8) as out_pool,
            tc.tile_pool(name="misc", bufs=1) as misc_pool,
            tc.tile_pool(name="psum", bufs=8, space="PSUM") as psum_pool,
        ):
            xT_v = xT[:]
            w_v = w[:]
            bias_v = biasb[:]
            at_v = at[:]
            bt_v = bt[:]
            out_v = out[:]

            # resident x^T tiles, DMA'd inside the n=0 loop as consumed;
            # n=0's W tiles stay resident too so the final m=6,7 re-sweep
            # needs no DMA at all.
            xts = [None] * (KO // XG)
            w0s = [None] * (KO // WG)
            a_sbs = [None] * (KO // AG)

            smask_sb = misc_pool.tile([J, T], F32, tag="smask")
            ams = misc_pool.tile([J, T], MMDT, tag="ams")
            b0_sb = misc_pool.tile([J, FD], MMDT, tag="b0")
            bias0_sb = misc_pool.tile([P, FD], F32, tag="bias0")

            NCH = T // FD  # a_allT token chunks (2)
            psa = [None] * NCH

            # n=0 splits m into (0..5) now + (6,7) last: the 2 a_allT PSUM
            # banks + 6 base banks fill PSUM during the first k-sweep.
            phases = (
                [(0, list(range(6)), True)]
                + [(n, list(range(MT)), False) for n in range(1, NT)]
                + [(0, [6], False), (0, [7], False)]
            )
            phases = phases * reps
            for n, ms, fuse_pro in phases:
                if n == 0:
                    b_sb, bias_sb = b0_sb, bias0_sb
                else:
                    b_sb = b_pool.tile([J, FD], MMDT, tag="b", name="b_sb")
                    nc.sync.dma_start(b_sb[:], bt_v[n])
                    bias_sb = bias_pool.tile([P, FD], F32, tag="bias", name="bias_sb")
                    nc.sync.dma_start(bias_sb[:], bias_v[n])
                if fuse_pro:
                    for c in range(NCH):
                        psa[c] = psum_pool.tile([P, FD], F32, tag="ps", name=f"psa_{c}")
                pss = {
                    m: psum_pool.tile([P, FD], F32, tag="ps", name=f"ps_{n}_{m}")
                    for m in ms
                }
                def _xt_dma(g):
                    xts[g] = xt_pool.tile(
                        [P, XG * T], MMDT, tag=f"xt{g}", name=f"xt{g}"
                    )
                    nc.sync.dma_start(xts[g][:], xT_v[g])

                def _w0_dma(g):
                    w0s[g] = w0_pool.tile(
                        [P, WG * FD], MMDT, tag=f"w0_{g}", name=f"w0_{g}"
                    )
                    nc.sync.dma_start(w0s[g][:], w_v[0, g])

                def _a_dma(g):
                    a_sbs[g] = a_pool.tile(
                        [P, AG * J], MMDT, tag="a", name="a_sb"
                    )
                    nc.sync.dma_start(a_sbs[g][:], at_v[g])

                for k in range(KO):
                    last_k = k == KO - 1
                    if fuse_pro:
                        if k == 0:
                            # startup: land k=0's operands first (smallest
                            # first), then the rest of block 0, then block-1
                            # prefetches; k>=31-only tensors go at k==AG
                            a_sbs[0] = a_pool.tile(
                                [P, AG * J], MMDT, tag="a", name="a_sb"
                            )
                            nc.sync.dma_start(a_sbs[0][:], at_v[0])
                            xts[0] = xt_pool.tile(
                                [P, XG * T], MMDT, tag="xt0", name="xt0"
                            )
                            nc.sync.dma_start(xts[0][:, 0:T], xT_v[0][:, 0:T])
                            w0s[0] = w0_pool.tile(
                                [P, WG * FD], MMDT, tag="w0_0", name="w0_0"
                            )
                            nc.sync.dma_start(w0s[0][:, 0:FD], w_v[0, 0][:, 0:FD])
                            nc.sync.dma_start(
                                xts[0][:, T:XG * T], xT_v[0][:, T:XG * T]
                            )
                            nc.sync.dma_start(
                                w0s[0][:, FD:WG * FD], w_v[0, 0][:, FD:WG * FD]
                            )
                            _xt_dma(1)
                            _w0_dma(1)
                            _a_dma(1)
                        else:
                            # prefetch one block ahead of first use
                            if k % XG == 0 and k // XG + 1 < KO // XG:
                                _xt_dma(k // XG + 1)
                            if k % WG == 0 and k // WG + 1 < KO // WG:
                                _w0_dma(k // WG + 1)
                            if k % AG == 0 and k // AG + 1 < KO // AG:
                                _a_dma(k // AG + 1)
                            if k == AG:
                                # k>=31-only tensors: issue behind the first
                                # few xT/W/A stream blocks
                                nc.sync.dma_start(smask_sb[:], smask[:])
                                nc.sync.dma_start(b0_sb[:], bt_v[0])
                                nc.sync.dma_start(bias0_sb[:], bias_v[0])
                    xk = xts[k // XG]
                    xo = (k % XG) * T
                    if n == 0:
                        wk = w0s[k // WG]
                    else:
                        if k % WG == 0:
                            wk = w_pool.tile(
                                [P, WG * FD], MMDT, tag="w", name="w_sb"
                            )
                            nc.sync.dma_start(wk[:], w_v[n, k // WG])
                    wo = (k % WG) * FD
                    if fuse_pro:
                        ak = a_sbs[k // AG]
                        ao = (k % AG) * J
                        for c in range(NCH):
                            nc.tensor.matmul(
                                psa[c][:], ak[:, ao:ao + J],
                                xk[:, xo + c * FD:xo + (c + 1) * FD],
                                start=(k == 0), stop=last_k,
                            )
                        if last_k:
                            for c in range(NCH):
                                nc.vector.tensor_mul(
                                    out=ams[:, c * FD:(c + 1) * FD],
                                    in0=psa[c][:],
                                    in1=smask_sb[:, c * FD:(c + 1) * FD],
                                )
                    for m in ms:
                        nc.tensor.matmul(
                            pss[m][:], xk[:, xo + m * P:xo + (m + 1) * P],
                            wk[:, wo:wo + FD],
                            start=(k == 0), stop=False,
                        )
                        if last_k:
                            # fused LoRA step + early staggered drain
                            nc.tensor.matmul(
                                pss[m][:], ams[:, m * P:(m + 1) * P], b_sb[:],
                                start=False, stop=True,
                            )
                            o_sb = out_pool.tile([P, FD], MMDT, tag="o", name="o_sb")
                            nc.vector.tensor_add(
                                out=o_sb[:], in0=pss[m][:], in1=bias_sb[:]
                            )
                            nc.sync.dma_start(
                                out_v[m * P:(m + 1) * P, n * FD:(n + 1) * FD],
                                o_sb[:],
                            )

    nc.compile()
    _NC_CACHE[key] = nc
    return nc


def _build_nc_sorted():
    """Variant for host-sorted tokens: each core's 1024 tokens span <=4
    consecutive adapters, so the LoRA-A prologue runs transposed with only
    JC=64 free-dim cycles per (m,k) instead of 512 per (chunk,k) — 16.4k
    PE cycles instead of 32.8k. The [tok, J] result is PE-transposed (2
    chunks per [128,128] transpose) back to the [J, tok] layout the fused
    LoRA-B step needs."""
    key = "nc_sorted"
    if key in _NC_CACHE:
        return _NC_CACHE[key]
    nc = bacc.Bacc(None, target_bir_lowering=False, debug=False)
    xT = nc.dram_tensor("xT", [KO // XG, P, XG * T], MMDT, kind="ExternalInput")
    w = nc.dram_tensor("w", [NT, KO // WG, P, WG * FD], MMDT, kind="ExternalInput")
    biasb = nc.dram_tensor("biasb", [NT, P, FD], F32, kind="ExternalInput")
    at = nc.dram_tensor("at", [KO // AG, P, AG * JC], MMDT, kind="ExternalInput")
    # window B rows duplicated to both partition halves so the LoRA rhs can
    # be sliced at partition 0 or 64 to match ams2's chunk placement
    bt = nc.dram_tensor("bt", [NT, 2 * JC, FD], MMDT, kind="ExternalInput")
    smaskT = nc.dram_tensor("smaskT", [P, MT * JC], F32, kind="ExternalInput")
    ident = nc.dram_tensor("ident", [P, P], MMDT, kind="ExternalInput")
    out = nc.dram_tensor("out", [T, D_OUT], MMDT, kind="ExternalOutput")

    with tile.TileContext(nc) as tc:
        with (
            tc.tile_pool(name="xt", bufs=1) as xt_pool,
            tc.tile_pool(name="w0", bufs=1) as w0_pool,
            tc.tile_pool(name="wp", bufs=4) as w_pool,
            tc.tile_pool(name="apool", bufs=3) as a_pool,
            tc.tile_pool(name="bp", bufs=2) as b_pool,
            tc.tile_pool(name="biasp", bufs=2) as bias_pool,
            tc.tile_pool(name="outp", bufs=8) as out_pool,
            tc.tile_pool(name="misc", bufs=1) as misc_pool,
            tc.tile_pool(name="psum", bufs=8, space="PSUM") as psum_pool,
        ):
            xT_v = xT[:]
            w_v = w[:]
            bias_v = biasb[:]
            at_v = at[:]
            bt_v = bt[:]
            out_v = out[:]

            xts = [None] * (KO // XG)
            w0s = [None] * (KO // WG)
            a_sbs = [None] * (KO // AG)

            smaskT_sb = misc_pool.tile([P, MT * JC], F32, tag="smaskT")
            amsT = misc_pool.tile([P, MT * JC], MMDT, tag="amsT")
            # ams2: chunk m lives at partitions (m%2)*JC.., cols (m//2)*P..
            ams2 = misc_pool.tile([P, (MT // 2) * P], MMDT, tag="ams2")
            ident_sb = misc_pool.tile([P, P], MMDT, tag="ident")
            b0_sb = misc_pool.tile([2 * JC, FD], MMDT, tag="b0")
            bias0_sb = misc_pool.tile([P, FD], F32, tag="bias0")

            def _ams_l(m):
                return ams2[
                    (m % 2) * JC:(m % 2) * JC + JC,
                    (m // 2) * P:(m // 2) * P + P,
                ]

            def _b_l(b_sb, m):
                return b_sb[(m % 2) * JC:(m % 2) * JC + JC, :]

            phases = (
                [(0, list(range(6)), True)]
                + [(n, list(range(MT)), False) for n in range(1, NT)]
                + [(0, [6], False), (0, [7], False)]
            )
            for n, ms, fuse_pro in phases:
                if n == 0:
                    b_sb, bias_sb = b0_sb, bias0_sb
                else:
                    b_sb = b_pool.tile([2 * JC, FD], MMDT, tag="b", name="b_sb")
                    nc.sync.dma_start(b_sb[:], bt_v[n])
                    bias_sb = bias_pool.tile([P, FD], F32, tag="bias", name="bias_sb")
                    nc.sync.dma_start(bias_sb[:], bias_v[n])
                if fuse_pro:
                    psaT = psum_pool.tile([P, MT * JC], F32, tag="ps", name="psaT")
                    pst = psum_pool.tile(
                        [P, (MT // 2) * P], MMDT, tag="ps", name="pst"
                    )
                pss = {
                    m: psum_pool.tile([P, FD], F32, tag="ps", name=f"ps_{n}_{m}")
                    for m in ms
                }

                def _xt_dma(g):
                    xts[g] = xt_pool.tile(
                        [P, XG * T], MMDT, tag=f"xt{g}", name=f"xt{g}"
                    )
                    nc.sync.dma_start(xts[g][:], xT_v[g])

                def _w0_dma(g):
                    w0s[g] = w0_pool.tile(
                        [P, WG * FD], MMDT, tag=f"w0_{g}", name=f"w0_{g}"
                    )
                    nc.sync.dma_start(w0s[g][:], w_v[0, g])

                def _a_dma(g):
                    a_sbs[g] = a_pool.tile(
                        [P, AG * JC], MMDT, tag=f"a{g}", name="a_sb"
                    )
                    nc.sync.dma_start(a_sbs[g][:], at_v[g])

                for k in range(KO):
                    last_k = k == KO - 1
                    if fuse_pro:
                        if k == 0:
                            # prologue runs at phase end now, so the base
                            # matmul operands (xT/W slivers) lead the queue
                            xts[0] = xt_pool.tile(
                                [P, XG * T], MMDT, tag="xt0", name="xt0"
                            )
                            nc.sync.dma_start(xts[0][:, 0:T], xT_v[0][:, 0:T])
                            w0s[0] = w0_pool.tile(
                                [P, WG * FD], MMDT, tag="w0_0", name="w0_0"
                            )
                            nc.sync.dma_start(w0s[0][:, 0:FD], w_v[0, 0][:, 0:FD])
                            a_sbs[0] = a_pool.tile(
                                [P, AG * JC], MMDT, tag="a0", name="a_sb"
                            )
                            nc.sync.dma_start(a_sbs[0][:], at_v[0])
                            nc.sync.dma_start(
                                xts[0][:, T:XG * T], xT_v[0][:, T:XG * T]
                            )
                            nc.sync.dma_start(
                                w0s[0][:, FD:WG * FD], w_v[0, 0][:, FD:WG * FD]
                            )
                            _xt_dma(1)
                            _w0_dma(1)
                            _a_dma(1)
                        else:
                            if k % XG == 0 and k // XG + 1 < KO // XG:
                                _xt_dma(k // XG + 1)
                            if k % WG == 0 and k // WG + 1 < KO // WG:
                                _w0_dma(k // WG + 1)
                            if k % AG == 0 and k // AG + 1 < KO // AG:
                                _a_dma(k // AG + 1)
                            if k == AG:
                                nc.sync.dma_start(smaskT_sb[:], smaskT[:])
                                nc.sync.dma_start(ident_sb[:], ident[:])
                                nc.sync.dma_start(b0_sb[:], bt_v[0])
                                nc.sync.dma_start(bias0_sb[:], bias_v[0])
                    xk = xts[k // XG]
                    xo = (k % XG) * T
                    if n == 0:
                        wk = w0s[k // WG]
                    else:
                        if k % WG == 0:
                            wk = w_pool.tile(
                                [P, WG * FD], MMDT, tag="w", name="w_sb"
                            )
                            nc.sync.dma_start(wk[:], w_v[n, k // WG])
                    wo = (k % WG) * FD
                    for m in ms:
                        nc.tensor.matmul(
                            pss[m][:], xk[:, xo + m * P:xo + (m + 1) * P],
                            wk[:, wo:wo + FD],
                            start=(k == 0), stop=False,
                        )
                        if last_k and not fuse_pro:
                            nc.tensor.matmul(
                                pss[m][:], _ams_l(m), _b_l(b_sb, m),
                                start=False, stop=True,
                            )
                            o_sb = out_pool.tile(
                                [P, FD], MMDT, tag="o", name="o_sb"
                            )
                            nc.vector.tensor_add(
                                out=o_sb[:], in0=pss[m][:], in1=bias_sb[:]
                            )
                            nc.sync.dma_start(
                                out_v[m * P:(m + 1) * P,
                                      n * FD:(n + 1) * FD],
                                o_sb[:],
                            )
                if fuse_pro:
                    # transposed LoRA-A prologue: xts are SBUF-resident now,
                    # so run each chunk as a time-contiguous accumulation
                    # group on psaT (interleaved groups on one PSUM tile
                    # clobber each other); masks/transposes pipeline on
                    # DVE/PE between chunks.
                    for m in range(MT):
                        for k in range(KO):
                            nc.tensor.matmul(
                                psaT[:, m * JC:(m + 1) * JC],
                                xts[k // XG][:, (k % XG) * T + m * P:
                                             (k % XG) * T + (m + 1) * P],
                                a_sbs[k // AG][:, (k % AG) * JC:
                                               (k % AG) * JC + JC],
                                start=(k == 0), stop=(k == KO - 1),
                            )
                        hs = slice(m * JC, (m + 1) * JC)
                        nc.vector.tensor_mul(
                            out=amsT[:, hs], in0=psaT[:, hs],
                            in1=smaskT_sb[:, hs],
                        )
                        if m % 2 == 1:
                            q = m // 2
                            nc.tensor.matmul(
                                pst[:, q * P:(q + 1) * P],
                                amsT[:, q * 2 * JC:(q + 1) * 2 * JC],
                                ident_sb[:],
                                is_transpose=True, start=True, stop=True,
                            )
                            nc.vector.tensor_copy(
                                ams2[:, q * P:(q + 1) * P],
                                pst[:, q * P:(q + 1) * P],
                            )
                    for m in ms:
                        nc.tensor.matmul(
                            pss[m][:], _ams_l(m), _b_l(b_sb, m),
                            start=False, stop=True,
                        )
                        o_sb = out_pool.tile([P, FD], MMDT, tag="o", name="o_sb")
                        nc.vector.tensor_add(
                            out=o_sb[:], in0=pss[m][:], in1=bias_sb[:]
                        )
                        nc.sync.dma_start(
                            out_v[m * P:(m + 1) * P, n * FD:(n + 1) * FD],
                            o_sb[:],
                        )

    nc.compile()
    _NC_CACHE[key] = nc
    return nc


def _prep_in_maps(x, weight, bias, A_buffer, B_buffer, scalings, token_indices):
    x = np.ascontiguousarray(np.asarray(x, np.float32))
    weight = np.asarray(weight, np.float32)
    bias = np.asarray(bias, np.float32)
    A_buffer = np.asarray(A_buffer, np.float32)
    B_buffer = np.asarray(B_buffer, np.float32)
    scalings = np.asarray(scalings, np.float32)
    token_indices = np.asarray(token_indices)

    xT_full = np.ascontiguousarray(x.T.astype(NP_MMDT))  # [D_IN, SEQ]
    # W packed so one DMA covers WG k-steps: [NT, KO//WG, P, WG*FD]
    w_t = np.ascontiguousarray(
        weight.reshape(KO // WG, WG, P, NT, FD)
        .transpose(3, 0, 2, 1, 4)
        .reshape(NT, KO // WG, P, WG * FD)
        .astype(NP_MMDT)
    )
    biasb = np.ascontiguousarray(
        np.broadcast_to(bias.reshape(NT, FD)[:, None, :], (NT, P, FD))
    )
    A_cat = A_buffer.reshape(J, D_IN)
    # A^T packed: [KO//AG, P, AG*J]
    at = np.ascontiguousarray(
        A_cat.T.reshape(KO // AG, AG, P, J)
        .transpose(0, 2, 1, 3)
        .reshape(KO // AG, P, AG * J)
        .astype(NP_MMDT)
    )
    bt = np.ascontiguousarray(
        B_buffer.transpose(0, 2, 1).reshape(J, NT, FD).transpose(1, 0, 2)
        .astype(NP_MMDT)
    )  # [NT, J, FD]
    adapter_of_row = (np.arange(J) // RANK).astype(token_indices.dtype)
    smask_full = (
        (token_indices[None, :] == adapter_of_row[:, None]).astype(np.float32)
        * scalings[None, :]
    )  # [J, SEQ]

    in_maps = []
    for c in range(N_CORES):
        sl = slice(c * T, (c + 1) * T)
        # xT shard packed: [KO//XG, P, XG*T]
        xT_c = np.ascontiguousarray(
            xT_full[:, sl]
            .reshape(KO // XG, XG, P, T)
            .transpose(0, 2, 1, 3)
            .reshape(KO // XG, P, XG * T)
        )
        in_maps.append({
            "xT": xT_c,
            "w": w_t,
            "biasb": biasb,
            "at": at,
            "bt": bt,
            "smask": np.ascontiguousarray(smask_full[:, sl]),
        })
    return in_maps


def _prep_in_maps_sorted(x, weight, bias, A_buffer, B_buffer, scalings,
                         token_indices):
    """Host-sorted variant: tokens globally sorted by adapter id, so each
    core's window spans <=4 consecutive adapters (JC=64 A/B rows). Returns
    (None, None) if some window exceeds 4 adapters (fall back to unsorted)."""
    x = np.ascontiguousarray(np.asarray(x, np.float32))
    weight = np.asarray(weight, np.float32)
    bias = np.asarray(bias, np.float32)
    A_buffer = np.asarray(A_buffer, np.float32)
    B_buffer = np.asarray(B_buffer, np.float32)
    scalings = np.asarray(scalings, np.float32)
    token_indices = np.asarray(token_indices)

    perm = np.argsort(token_indices, kind="stable")
    n_win = JC // RANK
    los = []
    for c in range(N_CORES):
        tok = token_indices[perm[c * T:(c + 1) * T]]
        lo = min(int(tok.min()), N_ADAPTERS - n_win)
        if int(tok.max()) >= lo + n_win:
            return None, None
        los.append(lo)

    xp = x[perm]
    sp = scalings[perm]
    tp = token_indices[perm]

    xT_full = np.ascontiguousarray(xp.T.astype(NP_MMDT))
    w_t = np.ascontiguousarray(
        weight.reshape(KO // WG, WG, P, NT, FD)
        .transpose(3, 0, 2, 1, 4)
        .reshape(NT, KO // WG, P, WG * FD)
        .astype(NP_MMDT)
    )
    biasb = np.ascontiguousarray(
        np.broadcast_to(bias.reshape(NT, FD)[:, None, :], (NT, P, FD))
    )
    A_cat = A_buffer.reshape(J, D_IN)
    B_catT = (
        B_buffer.transpose(0, 2, 1).reshape(J, NT, FD).transpose(1, 0, 2)
    )  # [NT, J, FD]
    ident = np.ascontiguousarray(np.eye(P, dtype=NP_MMDT))

    in_maps = []
    for c in range(N_CORES):
        sl = slice(c * T, (c + 1) * T)
        lo = los[c]
        rows = slice(lo * RANK, lo * RANK + JC)
        xT_c = np.ascontiguousarray(
            xT_full[:, sl]
            .reshape(KO // XG, XG, P, T)
            .transpose(0, 2, 1, 3)
            .reshape(KO // XG, P, XG * T)
        )
        at_c = np.ascontiguousarray(
            A_cat[rows].T
            .reshape(KO // AG, AG, P, JC)
            .transpose(0, 2, 1, 3)
            .reshape(KO // AG, P, AG * JC)
            .astype(NP_MMDT)
        )
        b_win = B_catT[:, rows].astype(NP_MMDT)  # [NT, JC, FD]
        bt_c = np.ascontiguousarray(
            np.concatenate([b_win, b_win], axis=1)
        )  # [NT, 2*JC, FD] — duplicated for partition-offset rhs slicing
        tok_c = tp[sl]
        adapter_of_col = lo + np.arange(JC) // RANK
        m_sm = (
            (tok_c[:, None] == adapter_of_col[None, :]).astype(np.float32)
            * sp[sl][:, None]
        )  # [T, JC]
        smT_c = np.ascontiguousarray(
            m_sm.reshape(MT, P, JC).transpose(1, 0, 2).reshape(P, MT * JC)
        )
        in_maps.append({
            "xT": xT_c,
            "w": w_t,
            "biasb": biasb,
            "at": at_c,
            "bt": bt_c,
            "smaskT": smT_c,
            "ident": ident,
        })
    return in_maps, perm


def _run(inputs, trace=False):
    prep = _prep_in_maps_sorted(**inputs)
    if prep[0] is not None:
        in_maps, perm = prep
        nc = _build_nc_sorted()
    else:
        nc = _build_nc()
        in_maps, perm = _prep_in_maps(**inputs), None
    res = run_bass_kernel_spmd(
        nc, in_maps, core_ids=list(range(N_CORES)), trace=trace
    )
    out = np.concatenate(
        [r["out"].astype(np.float32) for r in res.results], axis=0
    )
    if perm is not None:
        unsorted = np.empty_like(out)
        unsorted[perm] = out
        out = unsorted
    return out, res


def kernel(**inputs) -> np.ndarray:
    out, _ = _run(inputs, trace=False)
    return out


# revision 37
# speedup vs baseline: 1.0054x; 1.0054x over previous
"""Fused LoRA-Linear (per-token adapter routing) for 8 TRN2 NeuronCores.

Strategy:
  - Shard tokens: 8192 -> 1024 per core. Replicate weight/adapters.
    No cross-core communication (compute-regime problem).
  - Stack the 8 rank-16 adapters into one 128-row block:
        A_cat [128, 4096],  B_cat.T [128, 4096]
    Prologue per core: a_allT = A_cat @ x_shard^T  -> [128, 1024]  (PE)
    then ams = a_allT * smask where smask[j,t] = scal[t]*(idx[t]==j//16).
  - Main loop: out tile [128 tok, 512 dout] accumulates 32 base K-steps
    (lhsT = xT tile, rhs = W tile) plus ONE LoRA K-step
    (lhsT = ams column block, rhs = B_cat.T tile) in the same PSUM bank.
  - Drain: DVE adds broadcast bias while copying PSUM -> SBUF (bf16),
    DMA out; host converts back to f32.
  - Phase order: n=0 fuses the adapter prologue into its k-loop (PSUM:
    2 prologue + 6 base banks) so the x^T load streams concurrently with
    base matmuls; then n=1..7; the m=6 and m=7 re-sweeps of n=0 run LAST
    (m-outer, so m=6's drain hides under m=7's k-loop) on SBUF-resident
    W[0] tiles, hiding n=7's output-drain burst under their matmuls.
  - DMA batching: each DMA costs ~650ns SP-queue issue + ~625ns HWDGE +
    ~900ns semaphore propagation, so operands are host-packed into
    multi-k blocks (xT 2 k-steps, W/A 4 k-steps per DMA) and the n=0
    stream is issued one block ahead of first use; k>=31-only tensors
    (smask/B/bias) are issued behind the first two stream blocks.

Sorted variant (_build_nc_sorted, default path): the host globally sorts
tokens by adapter id (base GEMM is permutation-equivariant; output rows
un-permuted on host), so each core's 1024 tokens span <=4 consecutive
adapters and only a JC=64-row A/B window is needed. The LoRA-A prologue
then runs transposed — per (m,k) matmul free dim is JC=64 instead of
512 per (chunk,k), halving its PE cycles — as 8 time-contiguous
accumulation chunk-groups on one PSUM tile after the k-sweep (xts are
SBUF-resident; interleaved groups on one PSUM tile clobber each other),
with DVE masking and [128,128] PE transposes (2 chunks each, landing at
partition offsets 0/64 of one bank; B-window rows are duplicated to
both partition halves so the LoRA rhs base partition matches) pipelined
between chunks. Falls back to the unsorted builder for any input whose
sorted windows exceed 4 adapters.

Matmul operands are bf16 (fp32 PSUM accumulate): 1 PE cycle/row like
f32r but half the DMA bytes; ~3.0e-3 frobenius rel err vs the fp32
reference (harness gate is 2e-2).

Timeline-sim: 470.1us/core (sorted; unsorted fallback 472.6us) vs a
460.3us PE-busy floor; the rest is the startup DMA latency chain +
tail drain chain (fixed ~650+625+650+900ns per-DMA latencies).
"""

import numpy as np

import concourse.bass as bass
import concourse.bacc as bacc
import concourse.mybir as mybir
import concourse.tile as tile
from concourse.bass_utils import run_bass_kernel_spmd

SEQ, D_IN, D_OUT, RANK, N_ADAPTERS = 8192, 4096, 4096, 16, 8
N_CORES = 8
T = SEQ // N_CORES          # 1024 tokens per core
P = 128                     # partitions
FD = 512                    # matmul free dim (one PSUM bank)
KO = D_IN // P              # 32 contraction tiles
NT = D_OUT // FD            # 8 output column chunks
MT = T // P                 # 8 token tiles per core
J = N_ADAPTERS * RANK       # 128 stacked adapter rows
XG = 2                      # k-steps per xT DMA
WG = 4                      # k-steps per W DMA
AG = 4                      # k-steps per A DMA
F32 = mybir.dt.float32
MMDT = mybir.dt.bfloat16  # matmul operand dtype (bf16: full-rate PE, half DMA)
NP_MMDT = mybir.dt.np(MMDT)
JC = 64                     # sorted path: adapter-window rows per core (4x16)

_NC_CACHE = {}


def _build_nc(reps=1):
    # reps>1 repeats the whole program in one NEFF (benchmarking only:
    # wall(T_R) - wall(T_1) cancels identical RPC/dispatch overheads)
    key = f"nc{reps}"
    if key in _NC_CACHE:
        return _NC_CACHE[key]
    nc = bacc.Bacc(None, target_bir_lowering=False, debug=False)
    xT = nc.dram_tensor("xT", [KO // XG, P, XG * T], MMDT, kind="ExternalInput")
    w = nc.dram_tensor("w", [NT, KO // WG, P, WG * FD], MMDT, kind="ExternalInput")
    biasb = nc.dram_tensor("biasb", [NT, P, FD], F32, kind="ExternalInput")
    at = nc.dram_tensor("at", [KO // AG, P, AG * J], MMDT, kind="ExternalInput")
    bt = nc.dram_tensor("bt", [NT, J, FD], MMDT, kind="ExternalInput")
    smask = nc.dram_tensor("smask", [J, T], F32, kind="ExternalInput")
    # bf16 output (host converts back to f32): halves the drain DMA bytes
    out = nc.dram_tensor("out", [T, D_OUT], MMDT, kind="ExternalOutput")

    with tile.TileContext(nc) as tc:
        with (
            tc.tile_pool(name="xt", bufs=1) as xt_pool,
            tc.tile_pool(name="w0", bufs=1) as w0_pool,
            tc.tile_pool(name="wp", bufs=4) as w_pool,
            tc.tile_pool(name="apool", bufs=3) as a_pool,
            tc.tile_pool(name="bp", bufs=2) as b_pool,
            tc.tile_pool(name="biasp", bufs=2) as bias_pool,
            tc.tile_pool(name="outp", bufs=8) as out_pool,
            tc.tile_pool(name="misc", bufs=1) as misc_pool,
            tc.tile_pool(name="psum", bufs=8, space="PSUM") as psum_pool,
        ):
            xT_v = xT[:]
            w_v = w[:]
            bias_v = biasb[:]
            at_v = at[:]
            bt_v = bt[:]
            out_v = out[:]

            # resident x^T tiles, DMA'd inside the n=0 loop as consumed;
            # n=0's W tiles stay resident too so the final m=6,7 re-sweep
            # needs no DMA at all.
            xts = [None] * (KO // XG)
            w0s = [None] * (KO // WG)
            a_sbs = [None] * (KO // AG)

            smask_sb = misc_pool.tile([J, T], F32, tag="smask")
            ams = misc_pool.tile([J, T], MMDT, tag="ams")
            b0_sb = misc_pool.tile([J, FD], MMDT, tag="b0")
            bias0_sb = misc_pool.tile([P, FD], F32, tag="bias0")

            NCH = T // FD  # a_allT token chunks (2)
            psa = [None] * NCH

            # n=0 splits m into (0..5) now + (6,7) last: the 2 a_allT PSUM
            # banks + 6 base banks fill PSUM during the first k-sweep.
            phases = (
                [(0, list(range(6)), True)]
                + [(n, list(range(MT)), False) for n in range(1, NT)]
                + [(0, [6], False), (0, [7], False)]
            )
            phases = phases * reps
            for n, ms, fuse_pro in phases:
                if n == 0:
                    b_sb, bias_sb = b0_sb, bias0_sb
                else:
                    b_sb = b_pool.tile([J, FD], MMDT, tag="b", name="b_sb")
                    nc.sync.dma_start(b_sb[:], bt_v[n])
                    bias_sb = bias_pool.tile([P, FD], F32, tag="bias", name="bias_sb")
                    nc.sync.dma_start(bias_sb[:], bias_v[n])
                if fuse_pro:
                    for c in range(NCH):
                        psa[c] = psum_pool.tile([P, FD], F32, tag="ps", name=f"psa_{c}")
                pss = {
                    m: psum_pool.tile([P, FD], F32, tag="ps", name=f"ps_{n}_{m}")
                    for m in ms
                }
                def _xt_dma(g):
                    xts[g] = xt_pool.tile(
                        [P, XG * T], MMDT, tag=f"xt{g}", name=f"xt{g}"
                    )
                    nc.sync.dma_start(xts[g][:], xT_v[g])

                def _w0_dma(g):
                    w0s[g] = w0_pool.tile(
                        [P, WG * FD], MMDT, tag=f"w0_{g}", name=f"w0_{g}"
                    )
                    nc.sync.dma_start(w0s[g][:], w_v[0, g])

                def _a_dma(g):
                    a_sbs[g] = a_pool.tile(
                        [P, AG * J], MMDT, tag="a", name="a_sb"
                    )
                    nc.sync.dma_start(a_sbs[g][:], at_v[g])

                for k in range(KO):
                    last_k = k == KO - 1
                    if fuse_pro:
                        if k == 0:
                            # startup: land k=0's operands first (smallest
                            # first), then the rest of block 0, then block-1
                            # prefetches; k>=31-only tensors go at k==AG
                            a_sbs[0] = a_pool.tile(
                                [P, AG * J], MMDT, tag="a", name="a_sb"
                            )
                            nc.sync.dma_start(a_sbs[0][:], at_v[0])
                            xts[0] = xt_pool.tile(
                                [P, XG * T], MMDT, tag="xt0", name="xt0"
                            )
                            nc.sync.dma_start(xts[0][:, 0:T], xT_v[0][:, 0:T])
                            w0s[0] = w0_pool.tile(
                                [P, WG * FD], MMDT, tag="w0_0", name="w0_0"
                            )
                            nc.sync.dma_start(w0s[0][:, 0:FD], w_v[0, 0][:, 0:FD])
                            nc.sync.dma_start(
                                xts[0][:, T:XG * T], xT_v[0][:, T:XG * T]
                            )
                            nc.sync.dma_start(
                                w0s[0][:, FD:WG * FD], w_v[0, 0][:, FD:WG * FD]
                            )
                            _xt_dma(1)
                            _w0_dma(1)
                            _a_dma(1)
                        else:
                            # prefetch one block ahead of first use
                            if k % XG == 0 and k // XG + 1 < KO // XG:
                                _xt_dma(k // XG + 1)
                            if k % WG == 0 and k // WG + 1 < KO // WG:
                                _w0_dma(k // WG + 1)
                            if k % AG == 0 and k // AG + 1 < KO // AG:
                                _a_dma(k // AG + 1)
                            if k == AG:
                                # k>=31-only tensors: issue behind the first
                                # few xT/W/A stream blocks
                                nc.sync.dma_start(smask_sb[:], smask[:])
                                nc.sync.dma_start(b0_sb[:], bt_v[0])
                                nc.sync.dma_start(bias0_sb[:], bias_v[0])
                    xk = xts[k // XG]
                    xo = (k % XG) * T
                    if n == 0:
                        wk = w0s[k // WG]
                    else:
                        if k % WG == 0:
                            wk = w_pool.tile(
                                [P, WG * FD], MMDT, tag="w", name="w_sb"
                            )
                            nc.sync.dma_start(wk[:], w_v[n, k // WG])
                    wo = (k % WG) * FD
                    if fuse_pro:
                        ak = a_sbs[k // AG]
                        ao = (k % AG) * J
                        for c in range(NCH):
                            nc.tensor.matmul(
                                psa[c][:], ak[:, ao:ao + J],
                                xk[:, xo + c * FD:xo + (c + 1) * FD],
                                start=(k == 0), stop=last_k,
                            )
                        if last_k:
                            for c in range(NCH):
                                nc.vector.tensor_mul(
                                    out=ams[:, c * FD:(c + 1) * FD],
                                    in0=psa[c][:],
                                    in1=smask_sb[:, c * FD:(c + 1) * FD],
                                )
                    for m in ms:
                        nc.tensor.matmul(
                            pss[m][:], xk[:, xo + m * P:xo + (m + 1) * P],
                            wk[:, wo:wo + FD],
                            start=(k == 0), stop=False,
                        )
                        if last_k:
                            # fused LoRA step + early staggered drain
                            nc.tensor.matmul(
                                pss[m][:], ams[:, m * P:(m + 1) * P], b_sb[:],
                                start=False, stop=True,
                            )
                            o_sb = out_pool.tile([P, FD], MMDT, tag="o", name="o_sb")
                            nc.vector.tensor_add(
                                out=o_sb[:], in0=pss[m][:], in1=bias_sb[:]
                            )
                            nc.sync.dma_start(
                                out_v[m * P:(m + 1) * P, n * FD:(n + 1) * FD],
                                o_sb[:],
                            )

    nc.compile()
    _NC_CACHE[key] = nc
    return nc


def _build_nc_sorted():
    """Variant for host-sorted tokens: each core's 1024 tokens span <=4
    consecutive adapters, so the LoRA-A prologue runs transposed with only
    JC=64 free-dim cycles per (m,k) instead of 512 per (chunk,k) — 16.4k
    PE cycles instead of 32.8k. The [tok, J] result is PE-transposed (2
    chunks per [128,128] transpose) back to the [J, tok] layout the fused
    LoRA-B step needs."""
    key = "nc_sorted"
    if key in _NC_CACHE:
        return _NC_CACHE[key]
    nc = bacc.Bacc(None, target_bir_lowering=False, debug=False)
    xT = nc.dram_tensor("xT", [KO // XG, P, XG * T], MMDT, kind="ExternalInput")
    w = nc.dram_tensor("w", [NT, KO // WG, P, WG * FD], MMDT, kind="ExternalInput")
    biasb = nc.dram_tensor("biasb", [NT, P, FD], F32, kind="ExternalInput")
    at = nc.dram_tensor("at", [KO // AG, P, AG * JC], MMDT, kind="ExternalInput")
    # window B rows duplicated to both partition halves so the LoRA rhs can
    # be sliced at partition 0 or 64 to match ams2's chunk placement
    bt = nc.dram_tensor("bt", [NT, 2 * JC, FD], MMDT, kind="ExternalInput")
    smaskT = nc.dram_tensor("smaskT", [P, MT * JC], F32, kind="ExternalInput")
    ident = nc.dram_tensor("ident", [P, P], MMDT, kind="ExternalInput")
    out = nc.dram_tensor("out", [T, D_OUT], MMDT, kind="ExternalOutput")

    with tile.TileContext(nc) as tc:
        with (
            tc.tile_pool(name="xt", bufs=1) as xt_pool,
            tc.tile_pool(name="w0", bufs=1) as w0_pool,
            tc.tile_pool(name="wp", bufs=4) as w_pool,
            tc.tile_pool(name="apool", bufs=3) as a_pool,
            tc.tile_pool(name="bp", bufs=2) as b_pool,
            tc.tile_pool(name="biasp", bufs=2) as bias_pool,
            tc.tile_pool(name="outp", bufs=8) as out_pool,
            tc.tile_pool(name="misc", bufs=1) as misc_pool,
            tc.tile_pool(name="psum", bufs=8, space="PSUM") as psum_pool,
        ):
            xT_v = xT[:]
            w_v = w[:]
            bias_v = biasb[:]
            at_v = at[:]
            bt_v = bt[:]
            out_v = out[:]

            xts = [None] * (KO // XG)
            w0s = [None] * (KO // WG)
            a_sbs = [None] * (KO // AG)

            smaskT_sb = misc_pool.tile([P, MT * JC], F32, tag="smaskT")
            amsT = misc_pool.tile([P, MT * JC], MMDT, tag="amsT")
            # ams2: chunk m lives at partitions (m%2)*JC.., cols (m//2)*P..
            ams2 = misc_pool.tile([P, (MT // 2) * P], MMDT, tag="ams2")
            ident_sb = misc_pool.tile([P, P], MMDT, tag="ident")
            b0_sb = misc_pool.tile([2 * JC, FD], MMDT, tag="b0")
            bias0_sb = misc_pool.tile([P, FD], F32, tag="bias0")

            def _ams_l(m):
                return ams2[
                    (m % 2) * JC:(m % 2) * JC + JC,
                    (m // 2) * P:(m // 2) * P + P,
                ]

            def _b_l(b_sb, m):
                return b_sb[(m % 2) * JC:(m % 2) * JC + JC, :]

            phases = (
                [(0, list(range(6)), True)]
                + [(n, list(range(MT)), False) for n in range(1, NT)]
                + [(0, [6], False), (0, [7], False)]
            )
            for n, ms, fuse_pro in phases:
                if n == 0:
                    b_sb, bias_sb = b0_sb, bias0_sb
                else:
                    b_sb = b_pool.tile([2 * JC, FD], MMDT, tag="b", name="b_sb")
                    nc.sync.dma_start(b_sb[:], bt_v[n])
                    bias_sb = bias_pool.tile([P, FD], F32, tag="bias", name="bias_sb")
                    nc.sync.dma_start(bias_sb[:], bias_v[n])
                if fuse_pro:
                    psaT = psum_pool.tile([P, MT * JC], F32, tag="ps", name="psaT")
                    pst = psum_pool.tile(
                        [P, (MT // 2) * P], MMDT, tag="ps", name="pst"
                    )
                pss = {
                    m: psum_pool.tile([P, FD], F32, tag="ps", name=f"ps_{n}_{m}")
                    for m in ms
                }

                def _xt_dma(g):
                    xts[g] = xt_pool.tile(
                        [P, XG * T], MMDT, tag=f"xt{g}", name=f"xt{g}"
                    )
                    nc.sync.dma_start(xts[g][:], xT_v[g])

                def _w0_dma(g):
                    w0s[g] = w0_pool.tile(
                        [P, WG * FD], MMDT, tag=f"w0_{g}", name=f"w0_{g}"
                    )
                    nc.sync.dma_start(w0s[g][:], w_v[0, g])

                def _a_dma(g):
                    a_sbs[g] = a_pool.tile(
                        [P, AG * JC], MMDT, tag=f"a{g}", name="a_sb"
                    )
                    nc.sync.dma_start(a_sbs[g][:], at_v[g])

                for k in range(KO):
                    last_k = k == KO - 1
                    if fuse_pro:
                        if k == 0:
                            # prologue runs at phase end now, so the base
                            # matmul operands (xT/W slivers) lead the queue
                            xts[0] = xt_pool.tile(
                                [P, XG * T], MMDT, tag="xt0", name="xt0"
                            )
                            nc.sync.dma_start(xts[0][:, 0:T], xT_v[0][:, 0:T])
                            w0s[0] = w0_pool.tile(
                                [P, WG * FD], MMDT, tag="w0_0", name="w0_0"
                            )
                            nc.sync.dma_start(w0s[0][:, 0:FD], w_v[0, 0][:, 0:FD])
                            a_sbs[0] = a_pool.tile(
                                [P, AG * JC], MMDT, tag="a0", name="a_sb"
                            )
                            nc.sync.dma_start(a_sbs[0][:], at_v[0])
                            nc.sync.dma_start(
                                xts[0][:, T:XG * T], xT_v[0][:, T:XG * T]
                            )
                            nc.sync.dma_start(
                                w0s[0][:, FD:WG * FD], w_v[0, 0][:, FD:WG * FD]
                            )
                            _xt_dma(1)
                            _w0_dma(1)
                            _a_dma(1)
                        else:
                            if k % XG == 0 and k // XG + 1 < KO // XG:
                                _xt_dma(k // XG + 1)
                            if k % WG == 0 and k // WG + 1 < KO // WG:
                                _w0_dma(k // WG + 1)
                            if k % AG == 0 and k // AG + 1 < KO // AG:
                                _a_dma(k // AG + 1)
                            if k == AG:
                                nc.sync.dma_start(smaskT_sb[:], smaskT[:])
                                nc.sync.dma_start(ident_sb[:], ident[:])
                                nc.sync.dma_start(b0_sb[:], bt_v[0])
                                nc.sync.dma_start(bias0_sb[:], bias_v[0])
                    xk = xts[k // XG]
                    xo = (k % XG) * T
                    if n == 0:
                        wk = w0s[k // WG]
                    else:
                        if k % WG == 0:
                            wk = w_pool.tile(
                                [P, WG * FD], MMDT, tag="w", name="w_sb"
                            )
                            nc.sync.dma_start(wk[:], w_v[n, k // WG])
                    wo = (k % WG) * FD
                    for m in ms:
                        nc.tensor.matmul(
                            pss[m][:], xk[:, xo + m * P:xo + (m + 1) * P],
                            wk[:, wo:wo + FD],
                            start=(k == 0), stop=False,
                        )
                        if last_k and not fuse_pro:
                            nc.tensor.matmul(
                                pss[m][:], _ams_l(m), _b_l(b_sb, m),
                                start=False, stop=True,
                            )
                            o_sb = out_pool.tile(
                                [P, FD], MMDT, tag="o", name="o_sb"
                            )
                            nc.vector.tensor_add(
                                out=o_sb[:], in0=pss[m][:], in1=bias_sb[:]
                            )
                            nc.sync.dma_start(
                                out_v[m * P:(m + 1) * P,
                                      n * FD:(n + 1) * FD],
                                o_sb[:],
                            )
                if fuse_pro:
                    # transposed LoRA-A prologue: xts are SBUF-resident now,
                    # so run each chunk as a time-contiguous accumulation
                    # group on psaT (interleaved groups on one PSUM tile
                    # clobber each other); masks/transposes pipeline on
                    # DVE/PE between chunks.
                    for m in range(MT):
                        for k in range(KO):
                            nc.tensor.matmul(
                                psaT[:, m * JC:(m + 1) * JC],
                                xts[k // XG][:, (k % XG) * T + m * P:
                                             (k % XG) * T + (m + 1) * P],
                                a_sbs[k // AG][:, (k % AG) * JC:
                                               (k % AG) * JC + JC],
                                start=(k == 0), stop=(k == KO - 1),
                            )
                        hs = slice(m * JC, (m + 1) * JC)
                        nc.vector.tensor_mul(
                            out=amsT[:, hs], in0=psaT[:, hs],
                            in1=smaskT_sb[:, hs],
                        )
                        if m % 2 == 1:
                            q = m // 2
                            nc.tensor.matmul(
                                pst[:, q * P:(q + 1) * P],
                                amsT[:, q * 2 * JC:(q + 1) * 2 * JC],
                                ident_sb[:],
                                is_transpose=True, start=True, stop=True,
                            )
                            nc.vector.tensor_copy(
                                ams2[:, q * P:(q + 1) * P],
                                pst[:, q * P:(q + 1) * P],
                            )
                    for m in ms:
                        nc.tensor.matmul(
                            pss[m][:], _ams_l(m), _b_l(b_sb, m),
                            start=False, stop=True,
                        )
                        o_sb = out_pool.tile([P, FD], MMDT, tag="o", name="o_sb")
                        nc.vector.tensor_add(
                            out=o_sb[:], in0=pss[m][:], in1=bias_sb[:]
                        )
                        nc.sync.dma_start(
                            out_v[m * P:(m + 1) * P, n * FD:(n + 1) * FD],
                            o_sb[:],
                        )

    nc.compile()
    _NC_CACHE[key] = nc
    return nc


def _prep_in_maps(x, weight, bias, A_buffer, B_buffer, scalings, token_indices):
    x = np.ascontiguousarray(np.asarray(x, np.float32))
    weight = np.asarray(weight, np.float32)
    bias = np.asarray(bias, np.float32)
    A_buffer = np.asarray(A_buffer, np.float32)
    B_buffer = np.asarray(B_buffer, np.float32)
    scalings = np.asarray(scalings, np.float32)
    token_indices = np.asarray(token_indices)

    xT_full = np.ascontiguousarray(x.T.astype(NP_MMDT))  # [D_IN, SEQ]
    # W packed so one DMA covers WG k-steps: [NT, KO//WG, P, WG*FD]
    w_t = np.ascontiguousarray(
        weight.reshape(KO // WG, WG, P, NT, FD)
        .transpose(3, 0, 2, 1, 4)
        .reshape(NT, KO // WG, P, WG * FD)
        .astype(NP_MMDT)
    )
    biasb = np.ascontiguousarray(
        np.broadcast_to(bias.reshape(NT, FD)[:, None, :], (NT, P, FD))
    )
    A_cat = A_buffer.reshape(J, D_IN)
    # A^T packed: [KO//AG, P, AG*J]
    at = np.ascontiguousarray(
        A_cat.T.reshape(KO // AG, AG, P, J)
        .transpose(0, 2, 1, 3)
        .reshape(KO // AG, P, AG * J)
        .astype(NP_MMDT)
    )
    bt = np.ascontiguousarray(
        B_buffer.transpose(0, 2, 1).reshape(J, NT, FD).transpose(1, 0, 2)
        .astype(NP_MMDT)
    )  # [NT, J, FD]
    adapter_of_row = (np.arange(J) // RANK).astype(token_indices.dtype)
    smask_full = (
        (token_indices[None, :] == adapter_of_row[:, None]).astype(np.float32)
        * scalings[None, :]
    )  # [J, SEQ]

    in_maps = []
    for c in range(N_CORES):
        sl = slice(c * T, (c + 1) * T)
        # xT shard packed: [KO//XG, P, XG*T]
        xT_c = np.ascontiguousarray(
            xT_full[:, sl]
            .reshape(KO // XG, XG, P, T)
            .transpose(0, 2, 1, 3)
            .reshape(KO // XG, P, XG * T)
        )
        in_maps.append({
            "xT": xT_c,
            "w": w_t,
            "biasb": biasb,
            "at": at,
            "bt": bt,
            "smask": np.ascontiguousarray(smask_full[:, sl]),
        })
    return in_maps


def _prep_in_maps_sorted(x, weight, bias, A_buffer, B_buffer, scalings,
                         token_indices):
    """Host-sorted variant: tokens globally sorted by adapter id, so each
    core's window spans <=4 consecutive adapters (JC=64 A/B rows). Returns
    (None, None) if some window exceeds 4 adapters (fall back to unsorted)."""
    x = np.ascontiguousarray(np.asarray(x, np.float32))
    weight = np.asarray(weight, np.float32)
    bias = np.asarray(bias, np.float32)
    A_buffer = np.asarray(A_buffer, np.float32)
    B_buffer = np.asarray(B_buffer, np.float32)
    scalings = np.asarray(scalings, np.float32)
    token_indices = np.asarray(token_indices)

    perm = np.argsort(token_indices, kind="stable")
    n_win = JC // RANK
    los = []
    for c in range(N_CORES):
        tok = token_indices[perm[c * T:(c + 1) * T]]
        lo = min(int(tok.min()), N_ADAPTERS - n_win)
        if int(tok.max()) >= lo + n_win:
            return None, None
        los.append(lo)

    xp = x[perm]
    sp = scalings[perm]
    tp = token_indices[perm]

    xT_full = np.ascontiguousarray(xp.T.astype(NP_MMDT))
    w_t = np.ascontiguousarray(
        weight.reshape(KO // WG, WG, P, NT, FD)
        .transpose(3, 0, 2, 1, 4)
        .reshape(NT, KO // WG, P, WG * FD)
        .astype(NP_MMDT)
    )
    biasb = np.ascontiguousarray(
        np.broadcast_to(bias.reshape(NT, FD)[:, None, :], (NT, P, FD))
    )
    A_cat = A_buffer.reshape(J, D_IN)
    B_catT = (
        B_buffer.transpose(0, 2, 1).reshape(J, NT, FD).transpose(1, 0, 2)
    )  # [NT, J, FD]
    ident = np.ascontiguousarray(np.eye(P, dtype=NP_MMDT))

    in_maps = []
    for c in range(N_CORES):
        sl = slice(c * T, (c + 1) * T)
        lo = los[c]
        rows = slice(lo * RANK, lo * RANK + JC)
        xT_c = np.ascontiguousarray(
            xT_full[:, sl]
            .reshape(KO // XG, XG, P, T)
            .transpose(0, 2, 1, 3)
            .reshape(KO // XG, P, XG * T)
        )
        at_c = np.ascontiguousarray(
            A_cat[rows].T
            .reshape(KO // AG, AG, P, JC)
            .transpose(0, 2, 1, 3)
            .reshape(KO // AG, P, AG * JC)
            .astype(NP_MMDT)
        )
        b_win = B_catT[:, rows].astype(NP_MMDT)  # [NT, JC, FD]
        bt_c = np.ascontiguousarray(
            np.concatenate([b_win, b_win], axis=1)
        )  # [NT, 2*JC, FD] — duplicated for partition-offset rhs slicing
        tok_c = tp[sl]
        adapter_of_col = lo + np.arange(JC) // RANK
        m_sm = (
            (tok_c[:, None] == adapter_of_col[None, :]).astype(np.float32)
            * sp[sl][:, None]
        )  # [T, JC]
        smT_c = np.ascontiguousarray(
            m_sm.reshape(MT, P, JC).transpose(1, 0, 2).reshape(P, MT * JC)
        )
        in_maps.append({
            "xT": xT_c,
            "w": w_t,
            "biasb": biasb,
            "at": at_c,
            "bt": bt_c,
            "smaskT": smT_c,
            "ident": ident,
        })
    return in_maps, perm


def _run(inputs, trace=False):
    prep = _prep_in_maps_sorted(**inputs)
    if prep[0] is not None:
        in_maps, perm = prep
        nc = _build_nc_sorted()
    else:
        nc = _build_nc()
        in_maps, perm = _prep_in_maps(**inputs), None
    res = run_bass_kernel_spmd(
        nc, in_maps, core_ids=list(range(N_CORES)), trace=trace
    )
    out = np.concatenate(
        [r["out"].astype(np.float32) for r in res.results], axis=0
    )
    if perm is not None:
        unsorted = np.empty_like(out)
        unsorted[perm] = out
        out = unsorted
    return out, res


def kernel(**inputs) -> np.ndarray:
    out, _ = _run(inputs, trace=False)
    return out
